# revision 20
# baseline (speedup 1.0000x reference)
"""MAD predictor (retrieval_knn) — Trainium2 Bass/Tile kernel on 8 NeuronCores.

kernel(**inputs) takes the FULL inputs and returns the FULL (4096,) f32 output.
Sharding: batch edges split 512/core across the 8 cores; embeds replicated
(bf16); per-edge gathers of *inputs* (adjacency rows/cols, edge field rows,
x.g dots) are host-staged. Everything index-dependent on *computed* k-NN
samples runs on device.

Per core, per head h and build (src->dst, dst->src), per 128-edge row tile:
  S[b,n] = sum_{d<127} 2x_d e_d - |e_n|^2   ONE matmul per chunk: the norm
                                            bias rides in lhsT row 127
                                            against a constant-1 x row
                                            (MIPS-style fold; drops dim 127
                                            from the query side only, which
                                            jitters S by ~|2 x_127 e_127|
                                            << NN distance gaps).
  PSUM (f32) -> bf16 SBUF row via 2048-wide ACT copies.
  top-9 of each row = self + 8 nearest: DVE max8 over two 5000-wide blocks
  -> 16 candidates -> max8 + match_replace + max8 -> ranks 1..9; one
  FIND_INDEX8 over the full bf16 row resolves the 8 neighbor indices.
  Neighbor embedding rows gathered in ONE indirect DMA (8 rows/partition);
  adjacency bits in one indirect DMA from host-staged rows/cols.
  EG_k = e_s.g_b via gpsimd bcast-mult + DVE grouped reduce; x.g from host.
  d2_k = S_rank1 - S_k stashed; phase 3 (sqrt/exp/logits/softmin/sigmoid)
  runs once per row tile at the end, batched over all (h,bu) slots, so each
  activation table loads only a few times.
"""

import sys
from contextlib import ExitStack

for _p in ('/opt/trn_rl_repo', '/root/.axon_site/_ro/trn_rl_repo'):
    if _p not in sys.path:
        sys.path.append(_p)

import numpy as np
import ml_dtypes

import concourse.bass as bass
import concourse.bacc as bacc
import concourse.mybir as mybir
from concourse.tile import TileContext
from concourse.bass_utils import run_bass_kernel_spmd

BF16 = mybir.dt.bfloat16
F32 = mybir.dt.float32
U32 = mybir.dt.uint32
U8 = mybir.dt.uint8
P = 128
NEG_BIG = -3.0e38
bf = ml_dtypes.bfloat16

# problem constants (hardcoded per contract)
H, N, D = 4, 10000, 128
B, NCORES = 4096, 8
NB = B // NCORES          # 512 edges per core
RT = NB // P              # 4 row-tiles of 128 edges
NSENT = 8
STRIP = 2048              # PSUM strip (4 banks); matmuls fill it 512 at a time
MM = 512
SCAN_BLK = 5000           # max8 block (top-9 exact unless all 9 in one block)


def _chunks(total, step):
    out, o = [], 0
    while o < total:
        out.append((o, min(step, total - o)))
        o += step
    return out


def build_kernel(u):
    strips = _chunks(N, STRIP)
    blks = _chunks(N, SCAN_BLK)
    NSLOT = H * 2

    nc = bacc.Bacc("TRN2", target_bir_lowering=False, debug=False,
                   enable_asserts=True, num_devices=NCORES)

    eT = nc.declare_dram_parameter("eT", [H, D, N], BF16, isOutput=False)
    xT2 = nc.declare_dram_parameter("xT2", [H, 2, D, NB], BF16, isOutput=False)
    prow = nc.declare_dram_parameter("prow", [P, 1], U32, isOutput=False)
    joffs = nc.declare_dram_parameter("joffs", [P, 128], mybir.dt.uint16,
                                      isOutput=False)
    jdesc = nc.declare_dram_parameter("jdesc", [P, 128], BF16, isOutput=False)
    grows = nc.declare_dram_parameter("grows", [H, 2, NB, D], BF16, isOutput=False)
    xgh = nc.declare_dram_parameter("xgh", [H, 2, NB, 1], F32, isOutput=False)
    emb = {h: nc.declare_dram_parameter(f"emb_{h}", [N, D], BF16, isOutput=False)
           for h in range(H)}
    astage = {}
    for bu in range(2):
        for rt in range(RT):
            astage[(bu, rt)] = nc.declare_dram_parameter(
                f"astage_{bu}_{rt}", [P, N], U8, isOutput=False)
    out_p = nc.declare_dram_parameter("out", [NB, 1], F32, isOutput=True)

    with TileContext(nc) as tc, ExitStack() as ctx:
        pconst = ctx.enter_context(tc.tile_pool(name="const", bufs=1))
        pbig = ctx.enter_context(tc.tile_pool(name="big", bufs=2))
        psmall = ctx.enter_context(tc.tile_pool(name="small", bufs=4))
        pacc = ctx.enter_context(tc.tile_pool(name="acc", bufs=1))
        ppsum = ctx.enter_context(tc.tile_pool(name="psum", bufs=2, space="PSUM"))

        prow_t = pconst.tile([P, 1], U32)
        nc.sync.dma_start(out=prow_t[:], in_=prow[:, :])
        joffs_t = pconst.tile([P, 128], mybir.dt.uint16)
        nc.sync.dma_start(out=joffs_t[:], in_=joffs[:, :])
        jdesc_t = pconst.tile([P, 128], BF16)
        nc.sync.dma_start(out=jdesc_t[:], in_=jdesc[:, :])

        # PE warmup: sustained back-to-back matmuls un-throttle the HAM clock
        # gate (4/8 -> 8/8) before the real work starts.
        wsrc = pconst.tile([P, 512], BF16)
        nc.vector.memset(wsrc[:], 0.001)
        wps = ppsum.tile([P, STRIP], F32, tag="psS")
        for i in range(24):
            nc.tensor.matmul(wps[:, :MM], lhsT=wsrc[:, :P], rhs=wsrc[:],
                             start=True, stop=True)
        wout = pconst.tile([1, 1], F32)
        nc.vector.tensor_copy(wout[:], wps[:1, :1])

        # per row-tile stashes, filled slot-by-slot over the (h, bu) loops
        d2st = {rt: pacc.tile([P, NSLOT * 8], F32, tag=f"d2st{rt}", name=f"d2st{rt}")
                for rt in range(RT)}
        egst = {rt: pacc.tile([P, NSLOT * 8], F32, tag=f"egst{rt}", name=f"egst{rt}")
                for rt in range(RT)}
        a8st = {rt: pacc.tile([P, NSLOT * 8], U8, tag=f"a8st{rt}", name=f"a8st{rt}")
                for rt in range(RT)}
        xgst = {rt: pacc.tile([P, NSLOT], F32, tag=f"xgst{rt}", name=f"xgst{rt}")
                for rt in range(RT)}

        for h in range(H):
            eTh = pbig.tile([D, N], BF16, tag="eTh")
            nc.sync.dma_start(out=eTh[:], in_=eT[h])

            for bu in range(2):
                slot = h * 2 + bu
                xTs = psmall.tile([D, NB], BF16, tag="xTs")
                nc.sync.dma_start(out=xTs[:], in_=xT2[h, bu])

                for rt in range(RT):
                    nc.sync.dma_start(
                        out=xgst[rt][:, slot:slot + 1],
                        in_=xgh[h, bu, rt * P:(rt + 1) * P, :])

                    S_sb = pbig.tile([P, N], BF16, tag="S_sb", bufs=3)
                    for (so, sw) in strips:
                        psS = ppsum.tile([P, STRIP], F32, tag="psS")
                        for (co, cw) in _chunks(sw, MM):
                            nc.tensor.matmul(psS[:, co:co + cw],
                                             lhsT=xTs[:, rt * P:(rt + 1) * P],
                                             rhs=eTh[:, so + co:so + co + cw],
                                             start=True, stop=True)
                        nc.scalar.copy(S_sb[:, so:so + sw], psS[:, :sw])

                    # tournament fold 10000 -> 625 coset maxima (packed bf16
                    # tensor_tensor max runs in the DVE 2x mode, ~0.54 ns/col)
                    F1 = pbig.tile([P, 5000], BF16, tag="F1", bufs=3)
                    nc.vector.tensor_tensor(out=F1[:, 0:2500],
                                            in0=S_sb[:, 0:2500],
                                            in1=S_sb[:, 5000:7500],
                                            op=mybir.AluOpType.max)
                    nc.vector.tensor_tensor(out=F1[:, 2500:5000],
                                            in0=S_sb[:, 2500:5000],
                                            in1=S_sb[:, 7500:10000],
                                            op=mybir.AluOpType.max)
                    F2 = pbig.tile([P, 2500], BF16, tag="F2")
                    nc.vector.tensor_tensor(out=F2[:], in0=F1[:, 0:2500],
                                            in1=F1[:, 2500:5000],
                                            op=mybir.AluOpType.max)
                    F3 = pbig.tile([P, 1250], BF16, tag="F3")
                    nc.vector.tensor_tensor(out=F3[:], in0=F2[:, 0:1250],
                                            in1=F2[:, 1250:2500],
                                            op=mybir.AluOpType.max)
                    F4 = pbig.tile([P, 625], BF16, tag="F4")
                    nc.vector.tensor_tensor(out=F4[:], in0=F3[:, 0:625],
                                            in1=F3[:, 625:1250],
                                            op=mybir.AluOpType.max)

                    # top-9 coset maxima = self + 8 nearest (up to rare
                    # same-coset collisions among the top-9)
                    m1 = psmall.tile([P, 8], BF16, tag="m1")
                    nc.vector.max(out=m1[:], in_=F4[:])
                    candz = psmall.tile([P, 625], BF16, tag="candz")
                    nc.vector.match_replace(out=candz[:], in_to_replace=m1[:],
                                            in_values=F4[:], imm_value=NEG_BIG)
                    m2 = psmall.tile([P, 8], BF16, tag="m2")
                    nc.vector.max(out=m2[:], in_=candz[:])
                    nv = psmall.tile([P, 8], BF16, tag="nv")
                    nc.vector.tensor_copy(nv[:, 0:7], m1[:, 1:8])
                    nc.vector.tensor_copy(nv[:, 7:8], m2[:, 0:1])

                    # coset id of each neighbor value (625-wide scan), then
                    # resolve the exact column: gather the 16 coset members
                    # from S_sb (gpsimd SBUF gather) and match the value.
                    wq = psmall.tile([P, 8], mybir.dt.uint16, tag="wq")
                    nc.vector.max_index(wq[:], nv[:], F4[:])
                    cidx = psmall.tile([P, 128], mybir.dt.uint16, tag="cidx")
                    nc.vector.tensor_tensor(
                        out=cidx[:].rearrange("p (k j) -> p k j", k=8),
                        in0=wq[:].rearrange("p (k j) -> p k j", j=1
                                            ).to_broadcast([P, 8, 16]),
                        in1=joffs_t[:].rearrange("p (k j) -> p k j", k=8),
                        op=mybir.AluOpType.add)
                    cos16 = psmall.tile([P, 128], BF16, tag="cos16")
                    nc.gpsimd.indirect_copy(out=cos16[:], data=S_sb[:],
                                            idxs=cidx[:],
                                            i_know_ap_gather_is_preferred=True)
                    eqm = psmall.tile([P, 128], BF16, tag="eqm")
                    nc.vector.tensor_tensor(
                        out=eqm[:].rearrange("p (k j) -> p k j", k=8),
                        in0=cos16[:].rearrange("p (k j) -> p k j", k=8),
                        in1=nv[:].rearrange("p (k j) -> p k j", j=1
                                            ).to_broadcast([P, 8, 16]),
                        op=mybir.AluOpType.is_equal)
                    jm = psmall.tile([P, 128], BF16, tag="jm")
                    nc.vector.tensor_tensor(out=jm[:], in0=eqm[:],
                                            in1=jdesc_t[:],
                                            op=mybir.AluOpType.mult)
                    jinv = psmall.tile([P, 8], F32, tag="jinv")
                    nc.vector.tensor_reduce(jinv[:],
                                            jm[:].rearrange("p (k j) -> p k j",
                                                            k=8),
                                            axis=mybir.AxisListType.X,
                                            op=mybir.AluOpType.max)
                    # idx = wq + 625*(15 - jinv)
                    jneg = psmall.tile([P, 8], F32, tag="jneg")
                    nc.vector.tensor_scalar(out=jneg[:], in0=jinv[:],
                                            scalar1=-625.0, scalar2=9375.0,
                                            op0=mybir.AluOpType.mult,
                                            op1=mybir.AluOpType.add)
                    wqf = psmall.tile([P, 8], F32, tag="wqf")
                    nc.vector.tensor_copy(wqf[:], wq[:])
                    idxf = psmall.tile([P, 8], F32, tag="idxf")
                    nc.vector.tensor_tensor(out=idxf[:], in0=wqf[:], in1=jneg[:],
                                            op=mybir.AluOpType.add)
                    idx = psmall.tile([P, 8], U32, tag="idx")
                    nc.vector.tensor_scalar(out=idx[:], in0=idxf[:],
                                            scalar1=0.0, scalar2=None,
                                            op0=mybir.AluOpType.add)

                    # gather the 8 neighbor embedding rows in one call
                    erows = psmall.tile([P, 8 * D], BF16, tag="erows")
                    nc.gpsimd.indirect_dma_start(
                        out=erows[:], out_offset=None, in_=emb[h][:, :],
                        in_offset=bass.IndirectOffsetOnAxis(ap=idx[:, 0:8],
                                                            axis=0))

                    # adjacency bits at [s_k, other] from host-staged rows
                    eoff = psmall.tile([P, 8], U32, tag="eoff")
                    nc.vector.tensor_tensor(out=eoff[:], in0=idx[:],
                                            in1=prow_t[:].to_broadcast([P, 8]),
                                            op=mybir.AluOpType.add)
                    nc.gpsimd.indirect_dma_start(
                        out=a8st[rt][:, slot * 8:(slot + 1) * 8],
                        out_offset=None, in_=astage[(bu, rt)][:, :],
                        in_offset=bass.IndirectOffsetOnAxis(ap=eoff[:], axis=1))

                    # EG_k = e_s . g_b (gpsimd bcast-mult + DVE grouped reduce)
                    gtile = psmall.tile([P, D], BF16, tag="gtile")
                    nc.sync.dma_start(out=gtile[:],
                                      in_=grows[h, bu, rt * P:(rt + 1) * P, :])
                    prod = psmall.tile([P, 8 * D], F32, tag="prod")
                    e3 = erows[:].rearrange("p (o d) -> p o d", o=8)
                    g3 = gtile[:].rearrange("p (o d) -> p o d", o=1).to_broadcast(
                        [P, 8, D])
                    p3 = prod[:].rearrange("p (o d) -> p o d", o=8)
                    nc.gpsimd.tensor_tensor(out=p3, in0=e3, in1=g3,
                                            op=mybir.AluOpType.mult)
                    nc.vector.tensor_reduce(
                        egst[rt][:, slot * 8:(slot + 1) * 8], p3,
                        axis=mybir.AxisListType.X, op=mybir.AluOpType.add)

                    # stash S_k - S_rank1 = -d2_k (phase 3 takes sqrt(-x))
                    m1f = psmall.tile([P, 1], F32, tag="m1f")
                    nc.vector.tensor_copy(m1f[:], m1[:, 0:1])
                    nc.vector.tensor_scalar(
                        out=d2st[rt][:, slot * 8:(slot + 1) * 8],
                        in0=nv[:], scalar1=m1f[:, 0:1], scalar2=None,
                        op0=mybir.AluOpType.subtract)

        # phase 3, batched: per row tile, all (h, bu) slots at once
        dist = {}
        for rt in range(RT):
            dist[rt] = pacc.tile([P, NSLOT * 8], F32, tag=f"dist{rt}", name=f"dist{rt}")
            nc.scalar.activation(dist[rt][:], d2st[rt][:],
                                 mybir.ActivationFunctionType.Sqrt,
                                 scale=-1.0)
        w = {}
        for rt in range(RT):
            w[rt] = pacc.tile([P, NSLOT * 8], F32, tag=f"w{rt}", name=f"w{rt}")
            nc.scalar.activation(w[rt][:], dist[rt][:],
                                 mybir.ActivationFunctionType.Exp,
                                 bias=1.0, scale=-1.0)
        sig = {}
        for rt in range(RT):
            # l = (xg - eg) + u*(2*a - 1)
            l1 = psmall.tile([P, NSLOT * 8], F32, tag="l1")
            xg3 = xgst[rt][:].rearrange("p (o d) -> p o d", d=1).to_broadcast(
                [P, NSLOT, 8])
            eg3 = egst[rt][:].rearrange("p (o d) -> p o d", o=NSLOT)
            l13 = l1[:].rearrange("p (o d) -> p o d", o=NSLOT)
            nc.vector.tensor_tensor(out=l13, in0=xg3, in1=eg3,
                                    op=mybir.AluOpType.subtract)
            l2 = psmall.tile([P, NSLOT * 8], F32, tag="l2")
            nc.vector.scalar_tensor_tensor(
                out=l2[:], in0=a8st[rt][:], scalar=2.0 * u, in1=l1[:],
                op0=mybir.AluOpType.mult, op1=mybir.AluOpType.add)
            l3 = psmall.tile([P, NSLOT * 8], F32, tag="l3")
            nc.vector.tensor_scalar_add(l3[:], l2[:], -u)
            wl = psmall.tile([P, NSLOT * 8], F32, tag="wl")
            nc.vector.tensor_tensor(out=wl[:], in0=w[rt][:], in1=l3[:],
                                    op=mybir.AluOpType.mult)
            # per-slot sums over k, then per-head sums over bu
            sw8 = psmall.tile([P, NSLOT], F32, tag="sw8")
            nc.vector.tensor_reduce(sw8[:],
                                    w[rt][:].rearrange("p (o d) -> p o d",
                                                       o=NSLOT),
                                    axis=mybir.AxisListType.X,
                                    op=mybir.AluOpType.add)
            swl8 = psmall.tile([P, NSLOT], F32, tag="swl8")
            nc.vector.tensor_reduce(swl8[:],
                                    wl[:].rearrange("p (o d) -> p o d",
                                                    o=NSLOT),
                                    axis=mybir.AxisListType.X,
                                    op=mybir.AluOpType.add)
            swh = psmall.tile([P, H], F32, tag="swh")
            nc.vector.tensor_reduce(swh[:],
                                    sw8[:].rearrange("p (o d) -> p o d", o=H),
                                    axis=mybir.AxisListType.X,
                                    op=mybir.AluOpType.add)
            swlh = psmall.tile([P, H], F32, tag="swlh")
            nc.vector.tensor_reduce(swlh[:],
                                    swl8[:].rearrange("p (o d) -> p o d", o=H),
                                    axis=mybir.AxisListType.X,
                                    op=mybir.AluOpType.add)
            den = psmall.tile([P, H], F32, tag="den")
            nc.vector.tensor_scalar_add(den[:], swh[:], float(NSENT))
            rec = psmall.tile([P, H], F32, tag="rec")
            nc.vector.reciprocal(rec[:], den[:])
            smin = psmall.tile([P, H], F32, tag="smin")
            nc.vector.tensor_tensor(out=smin[:], in0=swlh[:], in1=rec[:],
                                    op=mybir.AluOpType.mult)
            acc = psmall.tile([P, 1], F32, tag="accf")
            nc.vector.tensor_reduce(acc[:],
                                    smin[:].rearrange("p (o d) -> p o d", o=1),
                                    axis=mybir.AxisListType.X,
                                    op=mybir.AluOpType.add)
            sig[rt] = psmall.tile([P, 1], F32, tag="sig", name=f"sig{rt}")
            nc.scalar.activation(sig[rt][:], acc[:],
                                 mybir.ActivationFunctionType.Sigmoid,
                                 scale=1.0 / H)
        for rt in range(RT):
            nc.sync.dma_start(out=out_p[rt * P:(rt + 1) * P, :], in_=sig[rt][:])

    nc.compile()
    return nc


def host_prep(embeds, field, uncertainty, adj, batch_edges):
    embeds = np.asarray(embeds, np.float32)
    field = np.asarray(field, np.float32)
    adj_u8 = (np.asarray(adj) != 0.0).astype(np.uint8)
    src = np.asarray(batch_edges[0]).astype(np.int64)
    dst = np.asarray(batch_edges[1]).astype(np.int64)

    # eT rows 0..126 = e_d, row 127 = -|e|^2 (full 128-dim norm)
    y2 = np.einsum('hnd,hnd->hn', embeds, embeds)          # (H, N) f32
    eTp = np.empty((H, D, N), dtype=bf)
    eTp[:, :D - 1, :] = embeds.transpose(0, 2, 1)[:, :D - 1, :].astype(bf)
    eTp[:, D - 1, :] = (-y2).astype(bf)
    emb_rows = [np.ascontiguousarray(embeds[hh]).astype(bf) for hh in range(H)]
    prow_np = (np.arange(P, dtype=np.uint32) * np.uint32(N)).reshape(P, 1)
    jj = np.arange(16)
    joffs_np = np.tile(np.tile(jj * 625, 8).astype(np.uint16), (P, 1))
    jdesc_np = np.tile(np.tile(15 - jj, 8).astype(bf), (P, 1))

    in_maps = []
    for m in range(NCORES):
        sl = slice(m * NB, (m + 1) * NB)
        s_sh, d_sh = src[sl], dst[sl]
        nodes = {0: s_sh, 1: d_sh}

        xT2 = np.empty((H, 2, D, NB), dtype=bf)
        grows_np = np.empty((H, 2, NB, D), dtype=bf)
        xg_np = np.empty((H, 2, NB, 1), dtype=np.float32)
        for bu in range(2):
            xb = embeds[:, nodes[bu], :]                   # (H, NB, D)
            gb = field[:, nodes[1 - bu], :]                # (H, NB, D)
            xT2[:, bu, :D - 1, :] = (2.0 * xb[:, :, :D - 1]
                                     ).transpose(0, 2, 1).astype(bf)
            xT2[:, bu, D - 1, :] = bf(1.0)
            grows_np[:, bu] = gb.astype(bf)
            xg_np[:, bu, :, 0] = np.einsum('hbd,hbd->hb', xb, gb)

        im = {"eT": eTp, "xT2": xT2, "prow": prow_np,
              "grows": grows_np, "xgh": xg_np,
              "joffs": joffs_np, "jdesc": jdesc_np}
        for hh in range(H):
            im[f"emb_{hh}"] = emb_rows[hh]
        for rt in range(RT):
            rsl = slice(rt * P, (rt + 1) * P)
            # build0 label: adj[s_k, dst_b] -> row p holds column adj[:, dst_p]
            im[f"astage_0_{rt}"] = np.ascontiguousarray(adj_u8[:, d_sh[rsl]].T)
            # build1 label: adj[src_b, s_k] -> row p holds row adj[src_p, :]
            im[f"astage_1_{rt}"] = np.ascontiguousarray(adj_u8[s_sh[rsl], :])
        in_maps.append(im)
    return in_maps


_CACHE = {}


def kernel(embeds, field, uncertainty, adj, batch_edges, _profile=None):
    """Full inputs in, full (4096,) f32 output. Runs on NeuronCores 0-7."""
    u = float(np.asarray(uncertainty).reshape(-1)[0])
    if ('nc', u) not in _CACHE:
        _CACHE[('nc', u)] = build_kernel(u)
    nc = _CACHE[('nc', u)]
    in_maps = host_prep(embeds, field, uncertainty, adj, batch_edges)
    res = run_bass_kernel_spmd(nc, in_maps, list(range(NCORES)),
                               trace=_profile is not None)
    if isinstance(_profile, dict):
        _profile['exec_time_ns'] = res.exec_time_ns
        _profile['res'] = res
    return np.concatenate([np.asarray(res.results[i]["out"], np.float32).reshape(-1)
                           for i in range(NCORES)])
